# revision 1
# baseline (speedup 1.0000x reference)
"""Trainium2 Bass kernel for nn_EntityCell (scatter_memory).

Math (per batch row b, entity e):
    gates = sigmoid(sum_d(s * (h + k)))              [B, E]
    h_tilda = tanh(h @ U + k @ V + (s @ W)[:, None]) [B, E, D]
    updated = h + gates[:, :, None] * h_tilda
    out = updated / sqrt(max(sum_d(updated^2), 1e-12))

Sharding: pure data parallel over the batch dim across 8 NeuronCores.

Per-core dataflow (B_loc=1024 rows; 4 load-pairs of 256 rows, each processed
as two 128-row compute chunks):
  - HWDGE DMA loads fp32 row-pairs; ScalarE casts each pair to fp16 in one op.
  - One DMA xbar transpose per tensor per chunk produces d-major fp16 tiles.
  - GpSimd computes (hT+kT) and *sT (TT-class ops never contend with DVE).
  - PE: per-entity matmuls hT_e@U + kT_e@V + sT@W accumulated in PSUM
    (fp16 in, fp32 accum); gate reduction via lhsT=t2T_e, rhs=ones.
  - ScalarE: tanh on 512-wide PSUM tiles -> fp16; sigmoid for gates.
  - VectorE: fused scalar_tensor_tensor update u = g*h_tilda + h, bn_stats
    based sum(u^2), Newton rsqrt (bit-trick seed), final scale to fp32.
"""

import numpy as np
from contextlib import nullcontext as _nullctx

B, E, D = 8192, 20, 128
N_CORES = 8
B_LOC = B // N_CORES
CHUNK = 128
N_PAIRS = B_LOC // (2 * CHUNK)
EG = 4  # entities per psum group (4*128 fp32 = one 2KB PSUM bank)

_CACHE = {}


def _build_nc(reps=1, loop_n=None, bf_bufs=2, tr_bufs=3, newton_iters=2,
              scale_on_act=0, ablate='full', store16=False, scale_ts=True,
              psm_bufs=7, psg_bufs=1, sm_bufs=4, k_pe_tr=True, h_pe_tr=False, psk_bufs=2, h_evac_act=False):
    import concourse.tile as tile
    from concourse import bacc, mybir
    from concourse.masks import make_identity
    from contextlib import ExitStack

    fp32 = mybir.dt.float32
    fp16 = mybir.dt.float16
    int32 = mybir.dt.int32
    AF = mybir.ActivationFunctionType
    OP = mybir.AluOpType

    nc = bacc.Bacc("TRN2", target_bir_lowering=False, debug=False)
    enc_d = nc.declare_dram_parameter("enc", [B_LOC, D], fp32, isOutput=False)
    prev_d = nc.declare_dram_parameter("prev", [B_LOC, E, D], fp32, isOutput=False)
    keys_d = nc.declare_dram_parameter("keys", [B_LOC, E, D], fp32, isOutput=False)
    u_d = nc.declare_dram_parameter("U", [D, D], fp32, isOutput=False)
    v_d = nc.declare_dram_parameter("V", [D, D], fp32, isOutput=False)
    w_d = nc.declare_dram_parameter("W", [D, D], fp32, isOutput=False)
    out_d = nc.declare_dram_parameter("out", [B_LOC, E, D], fp32, isOutput=True)

    # DRAM views: 256-row blocks as [pair_idx, partition, 2*E*D]
    prev_v = prev_d[:].rearrange("(n two p) e d -> n p two (e d)", two=2, p=CHUNK)
    keys_v = keys_d[:].rearrange("(n two p) e d -> n p two (e d)", two=2, p=CHUNK)
    enc_v = enc_d[:].rearrange("(n two p) d -> n p two d", two=2, p=CHUNK)
    out_v = out_d[:].rearrange("(n p) e d -> n p (e d)", p=CHUNK)

    with ExitStack() as ctx:
        tc = ctx.enter_context(tile.TileContext(nc))
        const_pool = ctx.enter_context(tc.tile_pool(name="const", bufs=1))
        io_pool = ctx.enter_context(tc.tile_pool(name="io", bufs=2))
        bf_pool = ctx.enter_context(tc.tile_pool(name="bf", bufs=bf_bufs))
        tr_pool = ctx.enter_context(tc.tile_pool(name="tr", bufs=tr_bufs))
        sm_pool = ctx.enter_context(tc.tile_pool(name="sm", bufs=sm_bufs))
        if k_pe_tr and psm_bufs > 5:
            psm_bufs = 5
        psm_pool = ctx.enter_context(tc.tile_pool(name="psm", bufs=psm_bufs, space="PSUM"))
        psg_pool = ctx.enter_context(tc.tile_pool(name="psg", bufs=psg_bufs, space="PSUM"))
        psk_pool = (
            ctx.enter_context(tc.tile_pool(name="psk", bufs=psk_bufs, space="PSUM"))
            if (k_pe_tr or h_pe_tr)
            else None
        )

        # ---- constants ----
        u32c = const_pool.tile([D, D], fp32)
        v32c = const_pool.tile([D, D], fp32)
        w32c = const_pool.tile([D, D], fp32)
        nc.sync.dma_start(u32c[:], u_d[:])
        nc.sync.dma_start(v32c[:], v_d[:])
        nc.sync.dma_start(w32c[:], w_d[:])
        u16c = const_pool.tile([D, D], fp16)
        v16c = const_pool.tile([D, D], fp16)
        w16c = const_pool.tile([D, D], fp16)
        nc.scalar.copy(u16c[:], u32c[:])
        nc.scalar.copy(v16c[:], v32c[:])
        nc.scalar.copy(w16c[:], w32c[:])
        ones16 = const_pool.tile([D, 1], fp16)
        nc.gpsimd.memset(ones16[:], 1.0)
        magic = const_pool.tile([CHUNK, E], int32)
        nc.gpsimd.memset(magic[:], 0x5F3759DF)
        if k_pe_tr or h_pe_tr:
            ident16 = const_pool.tile([D, D], fp16)
            make_identity(nc, ident16[:])

        loop_cm = (
            tc.For_i(0, loop_n, 1, hint_engines=tuple(mybir.ALL_ENGINES))
            if loop_n is not None
            else _nullctx()
        )
        with loop_cm:
         for cp in range(N_PAIRS * reps):
             n = cp % N_PAIRS
             # ---- paired loads (256 rows -> [128, 2, E, D]) ----
             h32p = io_pool.tile([CHUNK, 2, E, D], fp32, name="h32p")
             nc.sync.dma_start(h32p[:].rearrange("p a e d -> p a (e d)"), prev_v[n])
             k32p = io_pool.tile([CHUNK, 2, E, D], fp32, name="k32p")
             nc.sync.dma_start(k32p[:].rearrange("p a e d -> p a (e d)"), keys_v[n])
             s32p = io_pool.tile([CHUNK, 2, D], fp32, name="s32p")
             nc.sync.dma_start(s32p[:], enc_v[n])

             if ablate == 'dma':
                 for half in range(2):
                     nc.sync.dma_start(
                         out=out_v[2 * n + half],
                         in_=h32p[:, half].rearrange("p e d -> p (e d)"),
                     )
                 continue
             # ---- casts to fp16 (ScalarE), one op per pair ----
             h16p = bf_pool.tile([CHUNK, 2, E, D], fp16, name="h16p")
             nc.scalar.copy(h16p[:], h32p[:])
             k16p = bf_pool.tile([CHUNK, 2, E, D], fp16, name="k16p")
             nc.scalar.copy(k16p[:], k32p[:])
             s16p = bf_pool.tile([CHUNK, 2, D], fp16, name="s16p")
             nc.scalar.copy(s16p[:], s32p[:])

             for half in range(2):
                 c = 2 * n + half
                 h16 = h16p[:, half]
                 k16 = k16p[:, half]
                 s16 = s16p[:, half]

                 # ---- whole-tensor DMA xbar transposes to d-major ----
                 hT = tr_pool.tile([D, E, CHUNK], fp16, name="hT")
                 if h_pe_tr:
                     for gi in range(E // EG):
                         htp = psk_pool.tile([D, EG, CHUNK], fp16, name="htp",
                                             tag="ktp")
                         for j in range(EG):
                             nc.tensor.transpose(
                                 htp[:, j], h16[:, gi * EG + j], ident16[:]
                             )
                         if h_evac_act:
                             nc.scalar.copy(hT[:, gi * EG : (gi + 1) * EG], htp[:])
                         else:
                             nc.vector.tensor_copy(hT[:, gi * EG : (gi + 1) * EG], htp[:])
                 else:
                     nc.sync.dma_start_transpose(out=hT[:], in_=h16)
                 kT = tr_pool.tile([D, E, CHUNK], fp16, name="kT")
                 if k_pe_tr:
                     for gi in range(E // EG):
                         ktp = psk_pool.tile([D, EG, CHUNK], fp16, name="ktp")
                         for j in range(EG):
                             nc.tensor.transpose(
                                 ktp[:, j], k16[:, gi * EG + j], ident16[:]
                             )
                         nc.scalar.copy(kT[:, gi * EG : (gi + 1) * EG], ktp[:])
                 else:
                     nc.sync.dma_start_transpose(out=kT[:], in_=k16)
                 sT = tr_pool.tile([D, CHUNK], fp16, name="sT")
                 nc.sync.dma_start(out=sT[:], in_=s16, transpose=True)

                 if ablate == 'xpose':
                     nc.sync.dma_start(
                         out=out_v[c][:, : E * D // 2],
                         in_=hT[:].rearrange("p e d -> p (e d)").bitcast(fp32),
                     )
                     nc.sync.dma_start(
                         out=out_v[c][:, E * D // 2 :],
                         in_=kT[:].rearrange("p e d -> p (e d)").bitcast(fp32),
                     )
                     continue
                 # ---- gates input: t2T = (hT + kT) * sT  (GpSimd) ----
                 hkT = tr_pool.tile([D, E, CHUNK], fp16, name="hkT")
                 nc.gpsimd.tensor_tensor(hkT[:], hT[:], kT[:], OP.add)
                 sTb = sT[:].unsqueeze(1).broadcast_to([D, E, CHUNK])
                 t2T = hkT  # in-place: hkT is dead after this multiply
                 nc.gpsimd.tensor_tensor(t2T[:], hkT[:], sTb, OP.mult)

                 # ---- gates reduce over d on PE; sigmoid on ScalarE ----
                 gps = psg_pool.tile([CHUNK, E], fp32, name="gps")
                 for e in range(E):
                     nc.tensor.matmul(
                         gps[:, e : e + 1], t2T[:, e], ones16[:],
                         start=True, stop=True,
                     )
                 g32 = sm_pool.tile([CHUNK, E], fp32, name="g32")
                 nc.scalar.activation(g32[:], gps[:], AF.Sigmoid)

                 # ---- main matmuls + tanh ----
                 ht16 = bf_pool.tile([CHUNK, E, D], fp16, name="ht16")
                 for gi in range(E // EG):
                     ps = psm_pool.tile([CHUNK, EG, D], fp32, name="ps")
                     for j in range(EG):
                         e = gi * EG + j
                         nc.tensor.matmul(
                             ps[:, j], hT[:, e], u16c[:], start=True, stop=False
                         )
                         nc.tensor.matmul(
                             ps[:, j], kT[:, e], v16c[:], start=False, stop=False
                         )
                         nc.tensor.matmul(
                             ps[:, j], sT[:], w16c[:], start=False, stop=True
                         )
                     nc.scalar.activation(
                         ht16[:, gi * EG : (gi + 1) * EG], ps[:], AF.Tanh
                     )

                 if ablate == 'compute':
                     nc.sync.dma_start(
                         out=out_v[c][:, : E * D // 2],
                         in_=ht16[:].rearrange("p e d -> p (e d)").bitcast(fp32),
                     )
                     nc.sync.dma_start(
                         out=out_v[c][:, E * D // 2 :],
                         in_=t2T[:].rearrange("p e d -> p (e d)").bitcast(fp32),
                     )
                     continue
                 # ---- update u = g * h_tilda + h (VectorE, fused, in place
                 # over ht16: the tanh output is dead after this) ----
                 u16 = ht16
                 for e in range(E):
                     nc.vector.scalar_tensor_tensor(
                         u16[:, e], ht16[:, e], g32[:, e : e + 1], h16[:, e],
                         OP.mult, OP.add,
                     )

                 # ---- sum(u^2) via bn_stats (6 outputs/partition per call) ----
                 bn = sm_pool.tile([CHUNK, E, 6], fp32, name="bn")
                 for e in range(E):
                     nc.vector.bn_stats(bn[:, e, :], u16[:, e])
                 # normsq = 64*(mu_even^2 + mu_odd^2) + (cvar_even + cvar_odd)
                 t_a = sm_pool.tile([CHUNK, E], fp32, name="t_a")
                 nc.vector.tensor_tensor(t_a[:], bn[:, :, 1], bn[:, :, 1], OP.mult)
                 t_b = sm_pool.tile([CHUNK, E], fp32, name="t_b")
                 nc.vector.tensor_tensor(t_b[:], bn[:, :, 4], bn[:, :, 4], OP.mult)
                 t_ab = sm_pool.tile([CHUNK, E], fp32, name="t_ab")
                 nc.vector.tensor_tensor(t_ab[:], t_a[:], t_b[:], OP.add)
                 t_c = sm_pool.tile([CHUNK, E], fp32, name="t_c")
                 nc.vector.tensor_tensor(t_c[:], bn[:, :, 2], bn[:, :, 5], OP.add)
                 a32 = sm_pool.tile([CHUNK, E], fp32, name="a32")
                 nc.vector.scalar_tensor_tensor(
                     a32[:], t_ab[:], 64.0, t_c[:], OP.mult, OP.add
                 )
                 nc.vector.tensor_scalar(a32[:], a32[:], 1e-12, None, op0=OP.max)

                 # ---- r = rsqrt(a): bit-trick seed + Newton iterations ----
                 ti = sm_pool.tile([CHUNK, E], int32, name="ti")
                 nc.vector.tensor_scalar(
                     ti[:], a32[:].bitcast(int32), 1, None,
                     op0=OP.logical_shift_right,
                 )
                 yi = sm_pool.tile([CHUNK, E], int32, name="yi")
                 nc.vector.tensor_tensor(yi[:], magic[:], ti[:], OP.subtract)
                 y = yi[:].bitcast(fp32)
                 for _ in range(newton_iters):
                     y2 = sm_pool.tile([CHUNK, E], fp32, name="y2")
                     nc.vector.tensor_tensor(y2[:], y, y, OP.mult)
                     tt = sm_pool.tile([CHUNK, E], fp32, name="tt")
                     nc.vector.tensor_tensor(tt[:], a32[:], y2[:], OP.mult)
                     ww = sm_pool.tile([CHUNK, E], fp32, name="ww")
                     nc.vector.tensor_scalar(
                         ww[:], tt[:], -0.5, 1.5, op0=OP.mult, op1=OP.add
                     )
                     yn = sm_pool.tile([CHUNK, E], fp32, name="yn")
                     nc.vector.tensor_tensor(yn[:], y, ww[:], OP.mult)
                     y = yn[:]

                 # ---- scale out = u * r and store ----
                 if store16:
                     o16 = bf_pool.tile([CHUNK, E, D], fp16, name="o16")
                     for e in range(E):
                         nc.vector.scalar_tensor_tensor(
                             o16[:, e], u16[:, e], y[:, e : e + 1], u16[:, e],
                             OP.mult, OP.bypass,
                         )
                     nc.gpsimd.dma_start(
                         out=out_v[c], in_=o16[:].rearrange("p e d -> p (e d)")
                     )
                 else:
                     o32 = io_pool.tile([CHUNK, E, D], fp32, name="o32")
                     for e in range(E):
                         if e < scale_on_act:
                             nc.scalar.mul(o32[:, e], u16[:, e], y[:, e : e + 1])
                         elif scale_ts:
                             nc.vector.tensor_scalar(
                                 o32[:, e], u16[:, e], y[:, e : e + 1], None,
                                 op0=OP.mult,
                             )
                         else:
                             nc.vector.scalar_tensor_tensor(
                                 o32[:, e], u16[:, e], y[:, e : e + 1], u16[:, e],
                                 OP.mult, OP.bypass,
                             )
                     nc.sync.dma_start(
                         out=out_v[c], in_=o32[:].rearrange("p e d -> p (e d)")
                     )

    nc.compile()
    return nc


def _get_nc():
    if "nc" not in _CACHE:
        _CACHE["nc"] = _build_nc()
    return _CACHE["nc"]


def kernel(encoded_sents, prev_states, keys, U, V, W):
    import sys

    if "/opt/trn_rl_repo" not in sys.path:
        sys.path.insert(0, "/opt/trn_rl_repo")
    from concourse.bass_utils import run_bass_kernel_spmd

    nc = _get_nc()
    enc = np.ascontiguousarray(np.asarray(encoded_sents, dtype=np.float32))
    prev = np.ascontiguousarray(np.asarray(prev_states, dtype=np.float32))
    kys = np.ascontiguousarray(np.asarray(keys, dtype=np.float32))
    U = np.ascontiguousarray(np.asarray(U, dtype=np.float32))
    V = np.ascontiguousarray(np.asarray(V, dtype=np.float32))
    W = np.ascontiguousarray(np.asarray(W, dtype=np.float32))

    in_maps = []
    for i in range(N_CORES):
        lo, hi = i * B_LOC, (i + 1) * B_LOC
        in_maps.append(
            {
                "enc": enc[lo:hi],
                "prev": prev[lo:hi],
                "keys": kys[lo:hi],
                "U": U,
                "V": V,
                "W": W,
            }
        )

    res = run_bass_kernel_spmd(nc, in_maps, list(range(N_CORES)))
    out = np.concatenate([res.results[i]["out"] for i in range(N_CORES)], axis=0)
    return out.astype(np.float32)

